# revision 20
# baseline (speedup 1.0000x reference)
"""DiffusionGraphConv Trainium2 kernel (8-core SPMD, data-parallel over batch).

Math (halves the big-matmul FLOPs vs the reference order):
  reference: out[b,n,o] = sum_{f,m} mats_m[n,f,b] * W[f*5+m, o]
  with mats = [x0, s0 x0, 2 s0^2 x0 - x0, s1 x0, 2 s1^2 x0 - x0].
  Projection (width F=128 -> O=64) commutes with node-space diffusion:
    u_m = x0 @ W_m                      # [N, B_s*O] per core, cheap
    c0 = u1 + s0 @ (2 u2) ; c1 = u3 + s1 @ (2 u4)
    out = x0 @ (W0-W2-W4) + s0 @ c0 + s1 @ c1

Schedule (tuned against TimelineSim, which tracks this axon/fake-NRT device
within ~2%; modeled ~72.2 us/round vs a 71.7 us pure-PE floor):
  - All InstLdweights are stripped: every InstMatmult is self-loading
    (ldweights=True keeps the weights AP in ins[]), halving PE instructions.
  - The u1/u3 ("odd") projections never materialize: they are injected as
    64-col matmuls straight into the hop psums (psum scale 16x matches),
    so c = fp8(psum) costs a single rounding and the hop drain is one copy
    per 2-tile group. ONE start=True matmul per psum bank (the first DR
    chunk, full-width): multiple start=True matmuls in one bank reset the
    accumulation on this platform.
  - The u2/u4 ("even") projections are per-tile 1-bank psums (8 tiny
    matmuls + one 512-elem drain). PSUM-reading copies get no DVE 2x mode
    (f32 source), so they pace any phase whose PE work is short: the u4
    pass is interleaved into hop1 (which doesn't read u4) and the u2 pass
    for round r+1 into round r's final (which doesn't read u2), hiding all
    projection drains under long PE phases. A prologue covers round 0.
  - The final accumulates per node-tile: 8 DR (s0^T c0) + 8 v0 injections
    (v0 = x0 @ 4096*Wv0, 64-col writes) + 8 DR (s1^T c1) into a 1-bank
    psum, one drain copy, and ONE output DMA per round.
  - Support strips live RESIDENT in SBUF (16 MB fp8 for both), k-major, so
    hop lhsT chunks and final lhsT chunks are views of the same image.

Per-core, per-round: 1056 matmuls (512 fp8-DoubleRow diffusion + 256
injections + 256 projections + 32... see build), 56 copies, 1 DMA.
PE work: 512*256c (DR) + 256*64c (inject) + 256*64c (proj) + 128*64c (v0)
= 172k cycles = 71.7 us at 2.4 GHz; drains ride under PE phases.

Layouts (host-prepared "SBUF images"):
  x0t  [128 f, 16t*8b*128j] bf16: x0t[f, (t*8+b)*128+j] = cat(inputs,state)[b, t*128+j, f]
  wcat [128 f, 5*64]        bf16: [4096*(W0-W2-W4) | 2*W2/16 | 2*W4/16 | 16*W1 | 16*W3]
  s*t  [128 p, 8kp*2k2*2048n] fp8: s*t[p, ((kp*2+k2)*2048)+n] = SCALE*s[n, (kp*2+k2)*128+p]
       (k-major transposed strips: partition = k within 128-chunk)
  out  [2048 n, 8b*64o] bf16 = 4096 * out[b][n, o] (host divides by 4096)

Scales: strips at 256x (fp8), u-odd injections at 16x, u-even slots at 1/8x
so the hop psum lands at 16x; final psum = (16 c)*(256 s) = 4096x, v0
injected with host-prescaled 4096*Wv0; host divides by 4096 after readback.

Env quirks: walrus accepts <=1 sync-wait per instruction (_legalize_waits
hoists extras onto EventSemaphore carriers; simulators need legalize=False);
repeat=N re-runs the idempotent pipeline for wall-clock differencing since
this axon terminal has no NTFF profiling.
"""

import sys

if "/opt/trn_rl_repo" not in sys.path:
    sys.path.insert(0, "/opt/trn_rl_repo")

import numpy as np
import ml_dtypes

import concourse.bass as bass
import concourse.mybir as mybir
from concourse.tile import TileContext
from concourse.bass_utils import run_bass_kernel_spmd

BF16 = mybir.dt.bfloat16
FP8 = mybir.dt.float8e4
NPFP8 = ml_dtypes.float8_e4m3
SCALE = 256.0
F32 = mybir.dt.float32
NPBF16 = ml_dtypes.bfloat16

N = 2048          # graph nodes
F = 128           # input_size (64 input + 64 hidden)
B = 64            # global batch
NCORES = 8
BS = B // NCORES  # 8 batches per core
O = 64            # output features
NT = N // 128     # 16 node tiles
M5 = 5            # diffusion matrices
OBS = BS * O      # 512: width of diffusion operands per core


def _strip_ldweights(nc):
    """Remove the scheduler's InstLdweights and make each InstMatmult
    self-loading (ldweights=True): same semantics — the matmult's ins[] still
    carries the weights AP — at half the PE instruction count. Waits that
    lived on a removed ldweights are merged onto its matmult (then
    _legalize_waits re-legalizes)."""
    f = nc.m.functions[0]
    for blk in f.blocks:
        new = []
        pending = None
        for inst in blk.instructions:
            t = type(inst).__name__
            if t == "InstLdweights":
                si = inst.sync_info
                assert not (si and si.on_update)
                w = list(si.on_wait) if si else []
                if w:
                    pending = (pending or []) + w
                continue
            if t == "InstMatmult":
                inst.ldweights = True
                if pending:
                    si = inst.sync_info
                    inst.sync_info = mybir.SyncInfo(
                        on_wait=pending + (list(si.on_wait) if si else []),
                        on_update=list(si.on_update) if si else [],
                    )
                    pending = None
            new.append(inst)
        assert pending is None
        blk.instructions = new
    return nc


def _legalize_waits(nc, max_waits=1):
    """Walrus in this env encodes at most one sync-wait per instruction.

    Tile's sem assignment can emit 2-3 waits on one instruction; hoist the
    excess onto standalone EventSemaphore carriers (same engine, inserted
    just before), which the sequencer executes in order — semantics are
    identical, encoding is legal."""
    f = nc.m.functions[0]
    for blk in f.blocks:
        new_insts = []
        changed = False
        for inst in blk.instructions:
            si = inst.sync_info
            waits = list(si.on_wait) if si is not None else []
            if len(waits) > max_waits:
                for i, w in enumerate(waits[:-max_waits]):
                    ev = mybir.InstEventSemaphore(
                        name=f"{inst.name}-wsplit{i}",
                        engine=inst.engine,
                        ins=[],
                        outs=[],
                        sync_info=mybir.SyncInfo(on_wait=[w], on_update=[]),
                    )
                    new_insts.append(ev)
                inst.sync_info = mybir.SyncInfo(
                    on_wait=waits[-max_waits:], on_update=list(si.on_update)
                )
                changed = True
            new_insts.append(inst)
        if changed:
            blk.instructions = new_insts
    return nc


def build_bass(legalize=True, repeat=1, act_frac=0.5, debug_u=False):
    """Build the per-core SPMD Bass program.

    act_frac: fraction of drain copies routed to the Act engine (rest DVE).
    debug_u: add a "udbg" output dumping U_all after the last round.
    """
    nc = bass.Bass()
    x0t = nc.dram_tensor("x0t", [F, BS * N], BF16, kind="ExternalInput")
    wcat = nc.dram_tensor("wcat", [F, M5 * O], BF16, kind="ExternalInput")
    s0t = nc.dram_tensor("s0t", [128, NT * N], FP8, kind="ExternalInput")
    s1t = nc.dram_tensor("s1t", [128, NT * N], FP8, kind="ExternalInput")
    outp = nc.dram_tensor("out", [N, OBS], BF16, kind="ExternalOutput")
    udbg = (nc.dram_tensor("udbg", [128, 4 * NT * OBS], FP8, kind="ExternalOutput")
            if debug_u else None)

    with TileContext(nc) as tc:
        with (
            tc.tile_pool(name="persist", bufs=1) as persist,
            tc.tile_pool(name="pp", bufs=2, space="PSUM") as pp,
            tc.tile_pool(name="ppb", bufs=4, space="PSUM") as ppb,
        ):
            w_sb = persist.tile([F, M5 * O], BF16, name="w_sb")
            x0_sb = persist.tile([F, BS * N], BF16, name="x0_sb")
            nc.sync.dma_start(out=w_sb[:, :], in_=wcat[:, :])
            # x0 in halves so phase1's first tiles can start early
            half = BS * N // 2
            nc.sync.dma_start(out=x0_sb[:, 0:half], in_=x0t[:, 0:half])
            nc.sync.dma_start(out=x0_sb[:, half:], in_=x0t[:, half:])
            s0_sb = persist.tile([128, NT * N], FP8, name="s0_sb")
            s1_sb = persist.tile([128, NT * N], FP8, name="s1_sb")
            for sb_, st_ in ((s0_sb, s0t), (s1_sb, s1t)):
                for c in range(2):
                    lo, hi = c * NT * N // 2, (c + 1) * NT * N // 2
                    nc.sync.dma_start(out=sb_[:, lo:hi], in_=st_[:, lo:hi])

            # U_all: the four projection slots, [p, mi, kp, k2, b*o] fp8.
            # mi 0=u1->c0, 1=2*u2, 2=u3->c1, 3=2*u4.
            U_all = persist.tile([128, 4 * NT * OBS], FP8, name="u_all")
            o_sb = persist.tile([128, 4 * N], BF16, name="o_sb")

            def uview5():
                return U_all.rearrange(
                    "p (mi kp k2 b o) -> p mi kp k2 b o",
                    mi=4, kp=NT // 2, k2=2, b=BS,
                )

            def uslot(mi, kp):
                """[p, 2, OBS] rhs view: k-pair kp of slot mi (hop rhs)."""
                return U_all.rearrange(
                    "p (mi kp k2 c) -> p mi kp k2 c", mi=4, kp=NT // 2, k2=2
                )[:, mi, kp, :, :]

            def sview(sb):
                return sb.rearrange("p (kp k2 n) -> p kp k2 n", kp=NT // 2, k2=2)

            def x0view():
                return x0_sb.rearrange("p (t b j) -> p t b j", t=NT, b=BS)

            copy_eng = [
                lambda out, in_: nc.vector.tensor_copy(out=out, in_=in_),
                lambda out, in_: nc.scalar.copy(out=out, in_=in_),
            ]
            ci = [0]

            def copy(out, in_):
                # route act_frac of copies to Act (index 1), rest to DVE
                acc = int((ci[0] + 1) * act_frac) - int(ci[0] * act_frac)
                copy_eng[1 if acc else 0](out, in_)
                ci[0] += 1

            def proj_tile(t, mi_slot, wlo):
                # Project one node-tile into ONE even u-slot (u2 or u4):
                # 8 tiny matmuls (N=64) into a 1-bank psum + one 512-elem
                # drain copy. Interleaved into long PE phases (final / hop1)
                # so the psum-reading copies (no DVE 2x modes for f32
                # sources) ride the engine-idle windows.
                ps = ppb.tile([128, BS, O], F32, name="ps_pj", tag="ppb")
                for b in range(BS):
                    nc.tensor.matmul(
                        ps[:, b, :],
                        lhsT=x0_sb[:, (t * BS + b) * 128:(t * BS + b + 1) * 128],
                        rhs=w_sb[:, wlo:wlo + O],
                        start=True,
                        stop=True,
                    )
                du = uview5()[:, mi_slot, t // 2, t % 2, :, :]
                copy(du, ps[:, :, :])

            def hop(s_sb, src_mi, dst_mi, wlo, interleave):
                # 2 node-tiles per psum tile; per tile 8 DR matmuls plus 8
                # small matmuls injecting u_odd = x0 @ (16 W_odd) straight
                # into the accumulation (psum scale 16x matches: DR terms are
                # (256 s)(2u/16) = 16*(2 s u)). ONE copy per group drains
                # c = psum -> fp8 c-slot (single fp8 rounding for c).
                # `interleave(grp)` emits the piggybacked projection tiles.
                sv = sview(s_sb)
                for grp in range(NT // 2):
                    ps = pp.tile([128, 2, OBS], F32, name="ps_hop", tag="pp")
                    for tl in range(2):
                        t = grp * 2 + tl
                        # DR kp0 carries the single start=True (writes the
                        # full 512-col region — multiple start=True matmuls
                        # in one bank reset the accumulation); the u_odd
                        # injections then accumulate between the DR chunks.
                        for kp in range(NT // 2):
                            nc.tensor.matmul(
                                ps[:, tl, :],
                                lhsT=sv[:, kp, :, t * 128:(t + 1) * 128],
                                rhs=uslot(src_mi, kp),
                                start=(kp == 0),
                                stop=(kp == NT // 2 - 1),
                                perf_mode=mybir.MatmulPerfMode.DoubleRow,
                                skip_group_check=True,
                            )
                            if kp == 0:
                                for b in range(BS):
                                    nc.tensor.matmul(
                                        ps[:, tl, b * O:(b + 1) * O],
                                        lhsT=x0_sb[:, (t * BS + b) * 128:
                                                   (t * BS + b + 1) * 128],
                                        rhs=w_sb[:, wlo:wlo + O],
                                        start=False,
                                        stop=False,
                                        skip_group_check=True,
                                    )
                    d = U_all.rearrange(
                        "p (mi g c) -> p mi g c", mi=4, g=NT // 2
                    )[:, dst_mi, grp, :].rearrange("p (tl c) -> p tl c", tl=2)
                    copy(d, ps[:, :, :])
                    interleave(grp)

            def final(interleave):
                # Standard orientation: out[t*128+j, (b,o)] = 4096*out[b][n,o]
                # Per node-tile: 8 DR (s0^T c0) + 8 v0 injections (64-col
                # writes, half the PE cycles of the transposed form) + 8 DR
                # (s1^T c1) into a 1-bank psum; ONE drain; ONE output DMA.
                # `interleave(idx)` piggybacks next round's u2 projections.
                sv0, sv1 = sview(s0_sb), sview(s1_sb)
                for t in range(NT):
                    ps = pp.tile([128, OBS], F32, name="ps_fin", tag="pp")
                    for kp in range(NT // 2):
                        nc.tensor.matmul(
                            ps[:, :],
                            lhsT=sv0[:, kp, :, t * 128:(t + 1) * 128],
                            rhs=uslot(0, kp),
                            start=(kp == 0),
                            stop=False,
                            perf_mode=mybir.MatmulPerfMode.DoubleRow,
                            skip_group_check=True,
                        )
                    for b in range(BS):
                        nc.tensor.matmul(
                            ps[:, b * O:(b + 1) * O],
                            lhsT=x0_sb[:, (t * BS + b) * 128:(t * BS + b + 1) * 128],
                            rhs=w_sb[:, 0:O],
                            start=False,
                            stop=False,
                            skip_group_check=True,
                        )
                    for kp in range(NT // 2):
                        nc.tensor.matmul(
                            ps[:, :],
                            lhsT=sv1[:, kp, :, t * 128:(t + 1) * 128],
                            rhs=uslot(2, kp),
                            start=False,
                            stop=(kp == NT // 2 - 1),
                            perf_mode=mybir.MatmulPerfMode.DoubleRow,
                            skip_group_check=True,
                        )
                    dv = o_sb.rearrange("p (t c) -> p t c", t=NT)
                    copy(dv[:, t, :], ps[:, :])
                    interleave(t)
                nc.sync.dma_start(
                    out=outp.rearrange("(t p) c -> p t c", p=128),
                    in_=o_sb.rearrange("p (t c) -> p t c", t=NT),
                )

            def mk_interleave(mi_slot, wlo, per_call):
                def f(idx):
                    for t in range(idx * per_call, (idx + 1) * per_call):
                        if t < NT:
                            proj_tile(t, mi_slot, wlo)
                return f

            no_ilv = lambda idx: None

            # Round structure: u2 projections for round r ride inside round
            # r-1's final (prologue covers round 0); u4 projections ride
            # inside hop1 (hop2 needs them only later).
            for t in range(NT):
                proj_tile(t, 1, O)           # prologue: u2 for round 0
            for _rep in range(repeat):
                hop(s0_sb, 1, 0, 3 * O,      # c0 = u1 + s0 @ (2 u2)
                    mk_interleave(3, 2 * O, 2))   # + u4 projections
                hop(s1_sb, 3, 2, 4 * O, no_ilv)  # c1 = u3 + s1 @ (2 u4)
                final(mk_interleave(1, O, 1))    # + next round's u2
            if debug_u:
                nc.sync.dma_start(out=udbg[:, :], in_=U_all[:, :])
    _strip_ldweights(nc)
    return _legalize_waits(nc) if legalize else nc


_NC_CACHE = {}


def _get_nc():
    if "nc" not in _NC_CACHE:
        _NC_CACHE["nc"] = build_bass()
    return _NC_CACHE["nc"]


def make_inputs(support0, support1, inputs, state, weight):
    """Host-side layout prep -> per-core in_maps (shared replicated arrays)."""
    xs = np.concatenate(
        [
            np.asarray(inputs, np.float32).reshape(B, N, F // 2),
            np.asarray(state, np.float32).reshape(B, N, F // 2),
        ],
        axis=2,
    )  # [B, N, F]

    w = np.asarray(weight, np.float32).reshape(F, M5, O)
    # wv0 carries the final psum scale (SCALE*16 = 4096) so its injected
    # matmuls accumulate coherently with the DoubleRow terms.
    wv0 = (w[:, 0] - w[:, 2] - w[:, 4]) * (SCALE * 16.0)
    wcat = np.concatenate(
        [wv0, 2.0 * w[:, 2] / 16.0, 2.0 * w[:, 4] / 16.0,
         16.0 * w[:, 1], 16.0 * w[:, 3]], axis=1
    ).astype(NPBF16)  # [128, 320]: [v0 | 2W2/16 | 2W4/16 | 16W1 | 16W3]

    def strip_img(s):
        # k-major transposed strips: s*t[p, (kp*2+k2)*2048 + n]
        #   = fp8(SCALE * s[n, (kp*2+k2)*128 + p])
        r = (SCALE * np.asarray(s, np.float32)).astype(NPFP8)
        r = r.reshape(N, NT, 128).transpose(2, 1, 0)  # [p, k-chunk, n]
        return np.ascontiguousarray(r.reshape(128, NT * N))

    s0i, s1i = strip_img(support0), strip_img(support1)

    in_maps = []
    for c in range(NCORES):
        shard = xs[c * BS:(c + 1) * BS]                # [8b, N, F]
        # t-major SBUF image: x0t[f, t*BS*128 + b*128 + j] = shard[b, t*128+j, f]
        x0i = np.ascontiguousarray(
            shard.reshape(BS, NT, 128, F).transpose(3, 1, 0, 2).reshape(F, BS * N)
        ).astype(NPBF16)
        in_maps.append({"x0t": x0i, "wcat": wcat, "s0t": s0i, "s1t": s1i})
    return in_maps


def postprocess(results, biases):
    full = np.empty((B, N, O), np.float32)
    for c, r in enumerate(results):
        # out [t*128+j, b*64+o] = 4096 * out[b][n, o]
        r4 = r["out"].astype(np.float32).reshape(N, BS, O) * (1.0 / 4096.0)
        full[c * BS:(c + 1) * BS] = r4.transpose(1, 0, 2)
    full += np.asarray(biases, np.float32)[None, None, :]
    return full.reshape(B, N * O)


def kernel(support0, support1, inputs, state, weight, biases, output_size=None,
           **run_kwargs):
    nc = _get_nc()
    in_maps = make_inputs(support0, support1, inputs, state, weight)
    res = run_bass_kernel_spmd(nc, in_maps, core_ids=list(range(NCORES)),
                               **run_kwargs)
    out = postprocess(res.results, biases)
    if run_kwargs.get("trace"):
        return out, res
    return out


# revision 21
# speedup vs baseline: 1.6196x; 1.6196x over previous
"""DiffusionGraphConv Trainium2 kernel (8-core SPMD, data-parallel over batch).

Math (halves the big-matmul FLOPs vs the reference order):
  reference: out[b,n,o] = sum_{f,m} mats_m[n,f,b] * W[f*5+m, o]
  with mats = [x0, s0 x0, 2 s0^2 x0 - x0, s1 x0, 2 s1^2 x0 - x0].
  Projection (width F=128 -> O=64) commutes with node-space diffusion:
    u_m = x0 @ W_m                      # [N, B_s*O] per core, cheap
    c0 = u1 + s0 @ (2 u2) ; c1 = u3 + s1 @ (2 u4)
    out = x0 @ (W0-W2-W4) + s0 @ c0 + s1 @ c1

Schedule (tuned against TimelineSim, which tracks this axon/fake-NRT device
within ~2%; modeled ~72.2 us/round vs a 71.7 us pure-PE floor):
  - All InstLdweights are stripped: every InstMatmult is self-loading
    (ldweights=True keeps the weights AP in ins[]), halving PE instructions.
  - The u1/u3 ("odd") projections never materialize: they are injected as
    64-col matmuls straight into the hop psums (psum scale 16x matches),
    so c = fp8(psum) costs a single rounding and the hop drain is one copy
    per 2-tile group. ONE start=True matmul per psum bank (the first DR
    chunk, full-width): multiple start=True matmuls in one bank reset the
    accumulation on this platform.
  - The u2/u4 ("even") projections are per-tile 1-bank psums (8 tiny
    matmuls + one 512-elem drain). PSUM-reading copies get no DVE 2x mode
    (f32 source), so they pace any phase whose PE work is short: the u4
    pass is interleaved into hop1 (which doesn't read u4) and the u2 pass
    for round r+1 into round r's final (which doesn't read u2), hiding all
    projection drains under long PE phases. A prologue covers round 0.
  - The final accumulates per node-tile: 8 DR (s0^T c0) + 8 v0 injections
    (v0 = x0 @ 4096*Wv0, 64-col writes) + 8 DR (s1^T c1) into a 1-bank
    psum, one drain copy, and ONE output DMA per round.
  - Support strips live RESIDENT in SBUF (16 MB fp8 for both), k-major, so
    hop lhsT chunks and final lhsT chunks are views of the same image.

Per-core, per-round: 1152 matmuls (512 fp8-DoubleRow diffusion + 256
injections + 256 projections + 128 v0), 56 copies, 1 DMA.
PE work: 512*256c (DR) + 256*64c (inject) + 256*64c (proj) + 128*64c (v0)
= 172k cycles = 71.7 us at 2.4 GHz; drains ride under PE phases.

Layouts (host-prepared "SBUF images"):
  x0t  [128 f, 16t*8b*128j] bf16: x0t[f, (t*8+b)*128+j] = cat(inputs,state)[b, t*128+j, f]
  wcat [128 f, 5*64]        bf16: [4096*(W0-W2-W4) | 2*W2/16 | 2*W4/16 | 16*W1 | 16*W3]
  s*t  [128 p, 8kp*2k2*2048n] fp8: s*t[p, ((kp*2+k2)*2048)+n] = SCALE*s[n, (kp*2+k2)*128+p]
       (k-major transposed strips: partition = k within 128-chunk)
  out  [2048 n, 8b*64o] bf16 = 4096 * out[b][n, o] (host divides by 4096)

Scales: strips at 256x (fp8), u-odd injections at 16x, u-even slots at 1/8x
so the hop psum lands at 16x; final psum = (16 c)*(256 s) = 4096x, v0
injected with host-prescaled 4096*Wv0; host divides by 4096 after readback.

Env quirks: walrus accepts <=1 sync-wait per instruction (_legalize_waits
hoists extras onto EventSemaphore carriers; simulators need legalize=False);
repeat=N re-runs the idempotent pipeline for wall-clock differencing since
this axon terminal has no NTFF profiling.
"""

import sys

if "/opt/trn_rl_repo" not in sys.path:
    sys.path.insert(0, "/opt/trn_rl_repo")

import numpy as np
import ml_dtypes

import concourse.bass as bass
import concourse.mybir as mybir
from concourse.tile import TileContext
from concourse.bass_utils import run_bass_kernel_spmd

BF16 = mybir.dt.bfloat16
FP8 = mybir.dt.float8e4
NPFP8 = ml_dtypes.float8_e4m3
SCALE = 256.0
F32 = mybir.dt.float32
NPBF16 = ml_dtypes.bfloat16

N = 2048          # graph nodes
F = 128           # input_size (64 input + 64 hidden)
B = 64            # global batch
NCORES = 8
BS = B // NCORES  # 8 batches per core
O = 64            # output features
NT = N // 128     # 16 node tiles
M5 = 5            # diffusion matrices
OBS = BS * O      # 512: width of diffusion operands per core


def _strip_ldweights(nc):
    """Remove the scheduler's InstLdweights and make each InstMatmult
    self-loading (ldweights=True): same semantics — the matmult's ins[] still
    carries the weights AP — at half the PE instruction count. Waits that
    lived on a removed ldweights are merged onto its matmult (then
    _legalize_waits re-legalizes)."""
    f = nc.m.functions[0]
    for blk in f.blocks:
        new = []
        pending = None
        for inst in blk.instructions:
            t = type(inst).__name__
            if t == "InstLdweights":
                si = inst.sync_info
                assert not (si and si.on_update)
                w = list(si.on_wait) if si else []
                if w:
                    pending = (pending or []) + w
                continue
            if t == "InstMatmult":
                inst.ldweights = True
                if pending:
                    si = inst.sync_info
                    inst.sync_info = mybir.SyncInfo(
                        on_wait=pending + (list(si.on_wait) if si else []),
                        on_update=list(si.on_update) if si else [],
                    )
                    pending = None
            new.append(inst)
        assert pending is None
        blk.instructions = new
    return nc


def _legalize_waits(nc, max_waits=1):
    """Walrus in this env encodes at most one sync-wait per instruction.

    Tile's sem assignment can emit 2-3 waits on one instruction; hoist the
    excess onto standalone EventSemaphore carriers (same engine, inserted
    just before), which the sequencer executes in order — semantics are
    identical, encoding is legal."""
    f = nc.m.functions[0]
    for blk in f.blocks:
        new_insts = []
        changed = False
        for inst in blk.instructions:
            si = inst.sync_info
            waits = list(si.on_wait) if si is not None else []
            if len(waits) > max_waits:
                for i, w in enumerate(waits[:-max_waits]):
                    ev = mybir.InstEventSemaphore(
                        name=f"{inst.name}-wsplit{i}",
                        engine=inst.engine,
                        ins=[],
                        outs=[],
                        sync_info=mybir.SyncInfo(on_wait=[w], on_update=[]),
                    )
                    new_insts.append(ev)
                inst.sync_info = mybir.SyncInfo(
                    on_wait=waits[-max_waits:], on_update=list(si.on_update)
                )
                changed = True
            new_insts.append(inst)
        if changed:
            blk.instructions = new_insts
    return nc


def build_bass(legalize=True, repeat=1, act_frac=0.5, debug_u=False):
    """Build the per-core SPMD Bass program.

    act_frac: fraction of drain copies routed to the Act engine (rest DVE).
    debug_u: add a "udbg" output dumping U_all after the last round.
    """
    nc = bass.Bass()
    x0t = nc.dram_tensor("x0t", [F, BS * N], BF16, kind="ExternalInput")
    wcat = nc.dram_tensor("wcat", [F, M5 * O], BF16, kind="ExternalInput")
    s0t = nc.dram_tensor("s0t", [128, NT * N], FP8, kind="ExternalInput")
    s1t = nc.dram_tensor("s1t", [128, NT * N], FP8, kind="ExternalInput")
    outp = nc.dram_tensor("out", [N, OBS], BF16, kind="ExternalOutput")
    udbg = (nc.dram_tensor("udbg", [128, 4 * NT * OBS], FP8, kind="ExternalOutput")
            if debug_u else None)

    with TileContext(nc) as tc:
        with (
            tc.tile_pool(name="persist", bufs=1) as persist,
            tc.tile_pool(name="pp", bufs=2, space="PSUM") as pp,
            tc.tile_pool(name="ppb", bufs=4, space="PSUM") as ppb,
        ):
            w_sb = persist.tile([F, M5 * O], BF16, name="w_sb")
            x0_sb = persist.tile([F, BS * N], BF16, name="x0_sb")
            nc.sync.dma_start(out=w_sb[:, :], in_=wcat[:, :])
            # x0 in halves so phase1's first tiles can start early
            half = BS * N // 2
            nc.sync.dma_start(out=x0_sb[:, 0:half], in_=x0t[:, 0:half])
            nc.sync.dma_start(out=x0_sb[:, half:], in_=x0t[:, half:])
            s0_sb = persist.tile([128, NT * N], FP8, name="s0_sb")
            s1_sb = persist.tile([128, NT * N], FP8, name="s1_sb")
            for sb_, st_ in ((s0_sb, s0t), (s1_sb, s1t)):
                for c in range(2):
                    lo, hi = c * NT * N // 2, (c + 1) * NT * N // 2
                    nc.sync.dma_start(out=sb_[:, lo:hi], in_=st_[:, lo:hi])

            # U_all: the four projection slots, [p, mi, kp, k2, b*o] fp8.
            # mi 0=u1->c0, 1=2*u2, 2=u3->c1, 3=2*u4.
            U_all = persist.tile([128, 4 * NT * OBS], FP8, name="u_all")
            o_sb = persist.tile([128, 4 * N], BF16, name="o_sb")

            def uview5():
                return U_all.rearrange(
                    "p (mi kp k2 b o) -> p mi kp k2 b o",
                    mi=4, kp=NT // 2, k2=2, b=BS,
                )

            def uslot(mi, kp):
                """[p, 2, OBS] rhs view: k-pair kp of slot mi (hop rhs)."""
                return U_all.rearrange(
                    "p (mi kp k2 c) -> p mi kp k2 c", mi=4, kp=NT // 2, k2=2
                )[:, mi, kp, :, :]

            def sview(sb):
                return sb.rearrange("p (kp k2 n) -> p kp k2 n", kp=NT // 2, k2=2)

            def x0view():
                return x0_sb.rearrange("p (t b j) -> p t b j", t=NT, b=BS)

            copy_eng = [
                lambda out, in_: nc.vector.tensor_copy(out=out, in_=in_),
                lambda out, in_: nc.scalar.copy(out=out, in_=in_),
            ]
            ci = [0]

            def copy(out, in_):
                # route act_frac of copies to Act (index 1), rest to DVE
                acc = int((ci[0] + 1) * act_frac) - int(ci[0] * act_frac)
                copy_eng[1 if acc else 0](out, in_)
                ci[0] += 1

            def proj_tile(t, mi_slot, wlo):
                # Project one node-tile into ONE even u-slot (u2 or u4):
                # 8 tiny matmuls (N=64) into a 1-bank psum + one 512-elem
                # drain copy. Interleaved into long PE phases (final / hop1)
                # so the psum-reading copies (no DVE 2x modes for f32
                # sources) ride the engine-idle windows.
                ps = ppb.tile([128, BS, O], F32, name="ps_pj", tag="ppb")
                for b in range(BS):
                    nc.tensor.matmul(
                        ps[:, b, :],
                        lhsT=x0_sb[:, (t * BS + b) * 128:(t * BS + b + 1) * 128],
                        rhs=w_sb[:, wlo:wlo + O],
                        start=True,
                        stop=True,
                    )
                du = uview5()[:, mi_slot, t // 2, t % 2, :, :]
                copy(du, ps[:, :, :])

            def hop(s_sb, src_mi, dst_mi, wlo, interleave):
                # 2 node-tiles per psum tile; per tile 8 DR matmuls plus 8
                # small matmuls injecting u_odd = x0 @ (16 W_odd) straight
                # into the accumulation (psum scale 16x matches: DR terms are
                # (256 s)(2u/16) = 16*(2 s u)). ONE copy per group drains
                # c = psum -> fp8 c-slot (single fp8 rounding for c).
                # `interleave(grp)` emits the piggybacked projection tiles.
                sv = sview(s_sb)
                for grp in range(NT // 2):
                    ps = pp.tile([128, 2, OBS], F32, name="ps_hop", tag="pp")
                    for tl in range(2):
                        t = grp * 2 + tl
                        # DR kp0 carries the single start=True (writes the
                        # full 512-col region — multiple start=True matmuls
                        # in one bank reset the accumulation); the u_odd
                        # injections then accumulate between the DR chunks.
                        for kp in range(NT // 2):
                            nc.tensor.matmul(
                                ps[:, tl, :],
                                lhsT=sv[:, kp, :, t * 128:(t + 1) * 128],
                                rhs=uslot(src_mi, kp),
                                start=(kp == 0),
                                stop=(kp == NT // 2 - 1),
                                perf_mode=mybir.MatmulPerfMode.DoubleRow,
                                skip_group_check=True,
                            )
                            if kp == 0:
                                for b in range(BS):
                                    nc.tensor.matmul(
                                        ps[:, tl, b * O:(b + 1) * O],
                                        lhsT=x0_sb[:, (t * BS + b) * 128:
                                                   (t * BS + b + 1) * 128],
                                        rhs=w_sb[:, wlo:wlo + O],
                                        start=False,
                                        stop=False,
                                        skip_group_check=True,
                                    )
                    d = U_all.rearrange(
                        "p (mi g c) -> p mi g c", mi=4, g=NT // 2
                    )[:, dst_mi, grp, :].rearrange("p (tl c) -> p tl c", tl=2)
                    copy(d, ps[:, :, :])
                    interleave(grp)

            def final(interleave):
                # Standard orientation: out[t*128+j, (b,o)] = 4096*out[b][n,o]
                # Per node-tile: 8 DR (s0^T c0) + 8 v0 injections (64-col
                # writes, half the PE cycles of the transposed form) + 8 DR
                # (s1^T c1) into a 1-bank psum; ONE drain; ONE output DMA.
                # `interleave(idx)` piggybacks next round's u2 projections.
                sv0, sv1 = sview(s0_sb), sview(s1_sb)
                for t in range(NT):
                    ps = pp.tile([128, OBS], F32, name="ps_fin", tag="pp")
                    for kp in range(NT // 2):
                        nc.tensor.matmul(
                            ps[:, :],
                            lhsT=sv0[:, kp, :, t * 128:(t + 1) * 128],
                            rhs=uslot(0, kp),
                            start=(kp == 0),
                            stop=False,
                            perf_mode=mybir.MatmulPerfMode.DoubleRow,
                            skip_group_check=True,
                        )
                    for b in range(BS):
                        nc.tensor.matmul(
                            ps[:, b * O:(b + 1) * O],
                            lhsT=x0_sb[:, (t * BS + b) * 128:(t * BS + b + 1) * 128],
                            rhs=w_sb[:, 0:O],
                            start=False,
                            stop=False,
                            skip_group_check=True,
                        )
                    for kp in range(NT // 2):
                        nc.tensor.matmul(
                            ps[:, :],
                            lhsT=sv1[:, kp, :, t * 128:(t + 1) * 128],
                            rhs=uslot(2, kp),
                            start=False,
                            stop=(kp == NT // 2 - 1),
                            perf_mode=mybir.MatmulPerfMode.DoubleRow,
                            skip_group_check=True,
                        )
                    dv = o_sb.rearrange("p (t c) -> p t c", t=NT)
                    copy(dv[:, t, :], ps[:, :])
                    interleave(t)
                nc.sync.dma_start(
                    out=outp.rearrange("(t p) c -> p t c", p=128),
                    in_=o_sb.rearrange("p (t c) -> p t c", t=NT),
                )

            def mk_interleave(mi_slot, wlo, per_call):
                def f(idx):
                    for t in range(idx * per_call, (idx + 1) * per_call):
                        if t < NT:
                            proj_tile(t, mi_slot, wlo)
                return f

            no_ilv = lambda idx: None

            # Round structure: u2 projections for round r ride inside round
            # r-1's final (prologue covers round 0); u4 projections ride
            # inside hop1 (hop2 needs them only later).
            for t in range(NT):
                proj_tile(t, 1, O)           # prologue: u2 for round 0
            for _rep in range(repeat):
                hop(s0_sb, 1, 0, 3 * O,      # c0 = u1 + s0 @ (2 u2)
                    mk_interleave(3, 2 * O, 2))   # + u4 projections
                hop(s1_sb, 3, 2, 4 * O, no_ilv)  # c1 = u3 + s1 @ (2 u4)
                final(mk_interleave(1, O, 1))    # + next round's u2
            if debug_u:
                nc.sync.dma_start(out=udbg[:, :], in_=U_all[:, :])
    _strip_ldweights(nc)
    return _legalize_waits(nc) if legalize else nc


_NC_CACHE = {}


def _get_nc():
    if "nc" not in _NC_CACHE:
        _NC_CACHE["nc"] = build_bass()
    return _NC_CACHE["nc"]


def make_inputs(support0, support1, inputs, state, weight):
    """Host-side layout prep -> per-core in_maps (shared replicated arrays)."""
    xs = np.concatenate(
        [
            np.asarray(inputs, np.float32).reshape(B, N, F // 2),
            np.asarray(state, np.float32).reshape(B, N, F // 2),
        ],
        axis=2,
    )  # [B, N, F]

    w = np.asarray(weight, np.float32).reshape(F, M5, O)
    # wv0 carries the final psum scale (SCALE*16 = 4096) so its injected
    # matmuls accumulate coherently with the DoubleRow terms.
    wv0 = (w[:, 0] - w[:, 2] - w[:, 4]) * (SCALE * 16.0)
    wcat = np.concatenate(
        [wv0, 2.0 * w[:, 2] / 16.0, 2.0 * w[:, 4] / 16.0,
         16.0 * w[:, 1], 16.0 * w[:, 3]], axis=1
    ).astype(NPBF16)  # [128, 320]: [v0 | 2W2/16 | 2W4/16 | 16W1 | 16W3]

    def strip_img(s):
        # k-major transposed strips: s*t[p, (kp*2+k2)*2048 + n]
        #   = fp8(SCALE * s[n, (kp*2+k2)*128 + p])
        r = (SCALE * np.asarray(s, np.float32)).astype(NPFP8)
        r = r.reshape(N, NT, 128).transpose(2, 1, 0)  # [p, k-chunk, n]
        return np.ascontiguousarray(r.reshape(128, NT * N))

    s0i, s1i = strip_img(support0), strip_img(support1)

    in_maps = []
    for c in range(NCORES):
        shard = xs[c * BS:(c + 1) * BS]                # [8b, N, F]
        # t-major SBUF image: x0t[f, t*BS*128 + b*128 + j] = shard[b, t*128+j, f]
        x0i = np.ascontiguousarray(
            shard.reshape(BS, NT, 128, F).transpose(3, 1, 0, 2).reshape(F, BS * N)
        ).astype(NPBF16)
        in_maps.append({"x0t": x0i, "wcat": wcat, "s0t": s0i, "s1t": s1i})
    return in_maps


def postprocess(results, biases):
    full = np.empty((B, N, O), np.float32)
    for c, r in enumerate(results):
        # out [t*128+j, b*64+o] = 4096 * out[b][n, o]
        r4 = r["out"].astype(np.float32).reshape(N, BS, O) * (1.0 / 4096.0)
        full[c * BS:(c + 1) * BS] = r4.transpose(1, 0, 2)
    full += np.asarray(biases, np.float32)[None, None, :]
    return full.reshape(B, N * O)


def kernel(support0, support1, inputs, state, weight, biases, output_size=None,
           **run_kwargs):
    nc = _get_nc()
    in_maps = make_inputs(support0, support1, inputs, state, weight)
    res = run_bass_kernel_spmd(nc, in_maps, core_ids=list(range(NCORES)),
                               **run_kwargs)
    out = postprocess(res.results, biases)
    if run_kwargs.get("trace"):
        return out, res
    return out
